# revision 26
# baseline (speedup 1.0000x reference)
"""Causal multi-head attention block (QKV proj + causal softmax attention + out proj)
for Trainium2, sharded over 8 NeuronCores: data-parallel over batch (2), tensor-
parallel over heads (16 heads -> 4 per core).

Shapes (hardcoded): B=2, T=2048, C=1024, H=16, Dh=64.
Each core computes a partial output projection [T, C] for its 4 heads; the host
sums the 4 partials per batch and adds the fc bias.
"""

import os

import numpy as np

import concourse.bass as bass
import concourse.tile as tile
from concourse import bacc, mybir
from concourse.bass_utils import run_bass_kernel_spmd

F32 = mybir.dt.float32
BF16 = mybir.dt.bfloat16

B = 2
T = 2048
C = 1024
H_PER_CORE = 4  # local heads per core
DH = 64
O_CORE = H_PER_CORE * DH  # 256 output channels per core (per q/k/v)

TCH = 512  # t-chunk size (free dim of most matmuls)
N_CHUNKS = T // TCH  # 4
KT = T // 128  # 16 k-tiles of 128

_BUILD_CACHE = {}
LAST_RESULT = None


def build(t=T):
    n_chunks = t // TCH
    nc = bacc.Bacc("TRN2", target_bir_lowering=False)

    xT = nc.declare_dram_parameter("xT", [C, t], BF16, isOutput=False)
    wqkvT = nc.declare_dram_parameter("wqkvT", [C, 3 * O_CORE], BF16, isOutput=False)
    bqk = nc.declare_dram_parameter("bqk", [128, 4], F32, isOutput=False)
    bv_rep = nc.declare_dram_parameter("bv_rep", [128, O_CORE], F32, isOutput=False)
    wfcT = nc.declare_dram_parameter("wfcT", [O_CORE, C], BF16, isOutput=False)
    mask = nc.declare_dram_parameter("mask", [128, 128], BF16, isOutput=False)
    y = nc.declare_dram_parameter("y", [t, C], F32, isOutput=True)

    with (
        tile.TileContext(nc) as tc,
        tc.tile_pool(name="singles", bufs=1) as singles,
        tc.tile_pool(name="xpool", bufs=4) as xpool,
        tc.tile_pool(name="wtpool", bufs=6) as wtpool,
        tc.tile_pool(name="attnpool", bufs=3) as attnpool,
        tc.tile_pool(name="opool", bufs=3) as opool,
        tc.tile_pool(name="rpool", bufs=3) as rpool,
        tc.tile_pool(name="dpool", bufs=4, space="DRAM") as dpool,
        tc.tile_pool(name="mmps", bufs=2, space="PSUM") as mmps,
        tc.tile_pool(name="sps", bufs=2, space="PSUM") as sps,
        tc.tile_pool(name="avps", bufs=2, space="PSUM") as avps,
    ):
        # ---- persistent SBUF tensors ----
        xT_r0 = xT.rearrange("(co ci) t -> ci co t", ci=128)
        wq_sb = singles.tile([128, 8, 3 * O_CORE], BF16)  # [ci, co, o] = wqkvT
        wqkvT_r = wqkvT.rearrange("(co ci) o -> ci co o", ci=128)
        xt0 = xpool.tile([128, 8, TCH], BF16, tag="xt", name="xt0")
        for co in range(8):
            nc.sync.dma_start(xt0[:, co, :], xT_r0[:, co, :TCH])
            nc.sync.dma_start(wq_sb[:, co, :], wqkvT_r[:, co, :])
        wfc_sb = singles.tile([128, 2, C], BF16)  # [p, ks, n]
        nc.sync.dma_start(wfc_sb[:], wfcT.rearrange("(ks p) n -> p ks n", p=128))
        bqk_sb = singles.tile([128, 4], F32)
        nc.sync.dma_start(bqk_sb[:], bqk[:])
        bv_sb = singles.tile([128, H_PER_CORE, DH], F32)
        nc.sync.dma_start(bv_sb[:], bv_rep.rearrange("p (h d) -> p h d", h=H_PER_CORE))
        mask_sb = singles.tile([128, 128], BF16)
        nc.sync.dma_start(mask_sb[:], mask[:])

        qT_sb = singles.tile([128, 2, t], BF16)  # [dh + 64*(h%2), h//2, t]
        kT_sb = singles.tile([128, 2, t], BF16)
        v_sb = singles.tile([128, t // 128, H_PER_CORE, DH + 1], BF16)  # [k_in, kt, h, d|1]
        nc.vector.memset(v_sb[:, :, :, DH : DH + 1], 1.0)  # ones col -> softmax denom

        xT_r = xT.rearrange("(co ci) t -> ci co t", ci=128)

        def emit_fc(ts0, attn_t):
            for tt in range(4):
                for nn in range(2):
                    ps = mmps.tile([128, TCH], F32, tag="mm", name="fcps")
                    for ks in range(2):
                        nc.tensor.matmul(
                            ps[:],
                            attn_t[:, ks, tt * 128 : (tt + 1) * 128],
                            wfc_sb[:, ks, nn * TCH : (nn + 1) * TCH],
                            start=(ks == 0),
                            stop=(ks == 1),
                        )
                    ot = opool.tile([128, TCH], F32, tag="o")
                    nc.vector.tensor_copy(ot[:], ps[:])
                    nc.sync.dma_start(
                        y[ts0 + tt * 128 : ts0 + (tt + 1) * 128, nn * TCH : (nn + 1) * TCH],
                        ot[:],
                    )

        fcq = []
        for tcix in range(n_chunks):
            ts0 = tcix * TCH
            # ---- load x^T chunk ----
            if tcix == 0:
                xt = xt0
            else:
                xt = xpool.tile([128, 8, TCH], BF16, tag="xt")
                for co in range(8):
                    nc.sync.dma_start(xt[:, co, :], xT_r[:, co, ts0 : ts0 + TCH])

            # ---- Q^T / K^T projection: psum [o=128, t=512] ----
            for i in range(4):  # 0,1 -> q o-tiles; 2,3 -> k o-tiles
                ps = mmps.tile([128, TCH], F32, tag="mm")
                for co in range(8):
                    nc.tensor.matmul(
                        ps[:],
                        wq_sb[:, co, i * 128 : (i + 1) * 128],
                        xt[:, co, :],
                        start=(co == 0),
                        stop=(co == 7),
                    )
                dst = qT_sb if i < 2 else kT_sb
                nc.vector.tensor_scalar_add(
                    dst[:, i % 2, ts0 : ts0 + TCH], ps[:], bqk_sb[:, i : i + 1]
                )

            # ---- V projection: psum [t=128 (x2), d=256] ----
            for j in range(2):
                ps = mmps.tile([128, TCH], F32, tag="mm")
                for half in range(2):
                    tt = j * 2 + half
                    for co in range(8):
                        nc.tensor.matmul(
                            ps[:, half * 256 : (half + 1) * 256],
                            xt[:, co, tt * 128 : (tt + 1) * 128],
                            wq_sb[:, co, 2 * O_CORE : 3 * O_CORE],
                            start=(co == 0),
                            stop=(co == 7),
                        )
                kt0 = tcix * 4 + j * 2
                nc.vector.tensor_add(
                    v_sb[:, kt0 : kt0 + 2, :, 0:DH],
                    ps.rearrange("p (a h d) -> p a h d", a=2, h=H_PER_CORE),
                    bv_sb[:, None, :, :].to_broadcast((128, 2, H_PER_CORE, DH)),
                )

            # FC of the previous chunk goes here, overlapping this chunk's attention;
            # the last two FCs are deferred to fill the final (largest) attention.
            if fcq and tcix < n_chunks - 1:
                emit_fc(*fcq.pop(0))

            # ---- attention for this t-chunk: head pairs interleaved ----
            attn_t = attnpool.tile([128, 2, TCH], BF16, tag="attn")
            kimax = tcix * 4 + 3
            npairs = 2 * tcix + 2
            for hp in range(2):
                heads = (2 * hp, 2 * hp + 1)
                ps_av = {}
                for h in heads:
                    ps_av[h] = avps.tile([128, TCH], F32, tag="av", name=f"av{h}")
                wts = {}
                pend = []

                def do_av(item):
                    h, pj = item
                    wt = wts[(h, pj)]
                    for u in range(2):
                        ki = 2 * pj + u
                        sx = max(0, (ki - 4 * tcix) * 128)
                        nc.tensor.matmul(
                            ps_av[h][0 : DH + 1, sx:TCH],
                            v_sb[:, ki, h, :],
                            wt[:, u, sx:TCH],
                            start=(ki == 0),
                            stop=(ki == kimax),
                            skip_group_check=True,
                        )

                for pj in range(npairs):
                    for h in heads:
                        pb = (h % 2) * 64
                        ho = h // 2
                        st = sps.tile([128, 2, TCH], F32, tag="s")
                        for u in range(2):
                            ki = 2 * pj + u
                            nc.tensor.matmul(
                                st[:, u, :],
                                kT_sb[pb : pb + 64, ho, ki * 128 : (ki + 1) * 128],
                                qT_sb[pb : pb + 64, ho, ts0 : ts0 + TCH],
                                start=True,
                                stop=True,
                                skip_group_check=True,
                            )
                        wt = wtpool.tile([128, 2, TCH], BF16, tag="wt")
                        nc.scalar.activation(
                            wt[:],
                            st[:],
                            mybir.ActivationFunctionType.Exp,
                            scale=0.125,
                        )
                        for u in range(2):
                            ki = 2 * pj + u
                            m = ki - 4 * tcix
                            if m >= 0:
                                sx = m * 128
                                nc.vector.tensor_mul(
                                    wt[:, u, sx : sx + 128],
                                    wt[:, u, sx : sx + 128],
                                    mask_sb[:],
                                )
                        wts[(h, pj)] = wt
                        pend.append((h, pj))
                        if len(pend) > 2:
                            do_av(pend.pop(0))
                while pend:
                    do_av(pend.pop(0))

                for h in heads:
                    pb = (h % 2) * 64
                    ho = h // 2
                    a_sl = attn_t[pb : pb + 64, ho, :]
                    # free the AV psum quickly: unnormalized attn + denominator out
                    nc.vector.tensor_copy(a_sl, ps_av[h][0:DH, :])
                    den_sb = rpool.tile([1, TCH], F32, tag="den")
                    nc.vector.tensor_copy(den_sb[0:1, :], ps_av[h][DH : DH + 1, :])
                    # reciprocal on a [128, 4] reshape (DRAM bounce)
                    d1 = dpool.tile([1, TCH], F32)
                    nc.gpsimd.dma_start(d1[:], den_sb[0:1, :])
                    rp = rpool.tile([128, 4], F32, tag="rp")
                    nc.gpsimd.dma_start(
                        rp[:],
                        bass.AP(tensor=d1.tensor, offset=d1.offset, ap=[[4, 128], [1, 4]]),
                    )
                    rcp = rpool.tile([128, 4], F32, tag="rcp")
                    nc.vector.reciprocal(rcp[:], rp[:])
                    d2 = dpool.tile([1, TCH], F32)
                    nc.gpsimd.dma_start(
                        bass.AP(tensor=d2.tensor, offset=d2.offset, ap=[[4, 128], [1, 4]]),
                        rcp[:],
                    )
                    rep = rpool.tile([128, TCH], F32, tag="rep")
                    nc.gpsimd.dma_start(
                        rep[pb : pb + 64, :],
                        bass.AP(tensor=d2.tensor, offset=d2.offset, ap=[[0, 64], [1, TCH]]),
                    )
                    nc.vector.tensor_mul(a_sl, a_sl, rep[pb : pb + 64, :])

            fcq.append((ts0, attn_t))
        for item in fcq:
            emit_fc(*item)

    nc.compile()
    return nc


def _prep_core_inputs(x, w_qkv, b_qkv, w_fc, b_fc, core):
    b, g = core // 4, core % 4
    rq = slice(256 * g, 256 * g + 256)
    rk = slice(1024 + 256 * g, 1024 + 256 * g + 256)
    rv = slice(2048 + 256 * g, 2048 + 256 * g + 256)
    wcat = np.concatenate([w_qkv[rq], w_qkv[rk], w_qkv[rv]], axis=0)  # [768, 1024]
    bq, bk, bv = b_qkv[rq], b_qkv[rk], b_qkv[rv]
    import ml_dtypes

    bf16 = ml_dtypes.bfloat16
    return {
        "xT": np.ascontiguousarray(x[b].T).astype(bf16),
        "wqkvT": np.ascontiguousarray(wcat.T).astype(bf16),
        "bqk": np.ascontiguousarray(
            np.stack([bq[0:128], bq[128:256], bk[0:128], bk[128:256]], axis=1)
        ),
        "bv_rep": np.ascontiguousarray(np.broadcast_to(bv, (128, 256))),
        "wfcT": np.ascontiguousarray(w_fc[:, 256 * g : 256 * g + 256].T).astype(bf16),
        "mask": np.triu(np.ones((128, 128), dtype=np.float32)).astype(bf16),
    }


def kernel(x, w_qkv, b_qkv, w_fc, b_fc):
    global LAST_RESULT
    x = np.asarray(x, dtype=np.float32)
    w_qkv = np.asarray(w_qkv, dtype=np.float32)
    b_qkv = np.asarray(b_qkv, dtype=np.float32)
    w_fc = np.asarray(w_fc, dtype=np.float32)
    b_fc = np.asarray(b_fc, dtype=np.float32)

    if "nc" not in _BUILD_CACHE:
        _BUILD_CACHE["nc"] = build()
    nc = _BUILD_CACHE["nc"]

    in_maps = [
        _prep_core_inputs(x, w_qkv, b_qkv, w_fc, b_fc, core) for core in range(8)
    ]
    res = run_bass_kernel_spmd(
        nc,
        in_maps,
        core_ids=list(range(8)),
        trace=bool(os.environ.get("MHA_TRACE")),
    )
    LAST_RESULT = res

    out = np.empty((B, T, C), dtype=np.float32)
    for b in range(B):
        acc = res.results[4 * b]["y"].astype(np.float32)
        for g in range(1, 4):
            acc = acc + res.results[4 * b + g]["y"]
        out[b] = acc + b_fc
    return out
